# revision 8
# baseline (speedup 1.0000x reference)
# Trainium2 Bass kernel: LSTM greedy decoder (nn_Net_22565758173650)
#
# Strategy (8 cores, SPMD):
#  - hidden dim (768) tensor-parallel: 96 units/core for the LSTM cell
#  - vocab (50257) sharded: 6283 rows/core for the output projection + local argmax
#  - per step: AllGather(argmax candidates) -> token -> gather of host-precomputed
#    E_proj = emb @ W_ih.T + b_gate  (fp32-exact embedding+input-projection)
#    -> LSTM cell -> AllGather(h^T bf16) -> logits matmul (bf16, col-group tiled,
#    bias via hi/lo bf16 ones-rows) -> fused add+rowmax + max_index per window.
#
# bf16 single-pass matmuls with fp32 PSUM reproduce the reference argmax decisions
# exactly (min top-2 logit gap 5.4e-4 vs ~2e-5 perturbation); logits rel err ~1e-3.

import numpy as np
import ml_dtypes

B = 64          # batch
T = 32          # decode steps
HID = 768
VOCAB = 50257
NC = 8
HS = HID // NC            # 96 hidden units per core
VS = -(-VOCAB // NC)      # 6283 vocab rows per core (last core padded)
VPAD = NC * VS            # 50264
KC = HID // 128           # 6 contraction chunks
G4 = 4 * HS               # 384 gate rows per core
SOS = 1
NEG = -1e30
WIN = 512                 # logits N-window (one PSUM bank)
NWIN = -(-VS // WIN)      # 13 windows
F32 = np.float32
BF16 = ml_dtypes.bfloat16

_cache = {}


def _build(t_steps):
    import concourse.bacc as bacc
    import concourse.bass as bass
    import concourse.mybir as mybir
    import concourse.tile as tile

    dt = mybir.dt
    Alu = mybir.AluOpType
    Act = mybir.ActivationFunctionType

    nc = bacc.Bacc("TRN2", target_bir_lowering=False, debug=False,
                   enable_asserts=False, num_devices=NC)

    # ---- I/O ----
    eproj = nc.dram_tensor("eproj", [VOCAB, G4], dt.float32, kind="ExternalInput").ap()
    whhT = nc.dram_tensor("whhT", [HID, G4], dt.bfloat16, kind="ExternalInput").ap()
    woutT = nc.dram_tensor("woutT", [HID, VS], dt.bfloat16, kind="ExternalInput").ap()
    bias2 = nc.dram_tensor("bias2", [2, VS], dt.bfloat16, kind="ExternalInput").ap()
    ones2 = nc.dram_tensor("ones2", [2, B], dt.bfloat16, kind="ExternalInput").ap()
    ident = nc.dram_tensor("ident", [B, B], dt.float32, kind="ExternalInput").ap()
    cbase = nc.dram_tensor("cbase", [B, NWIN], dt.float32, kind="ExternalInput").ap()

    logits_o = nc.dram_tensor("logits_s", [B, t_steps, VS], dt.float32,
                              kind="ExternalOutput").ap()
    h_o = nc.dram_tensor("h_s", [B, HS], dt.float32, kind="ExternalOutput").ap()
    c_o = nc.dram_tensor("c_s", [B, HS], dt.float32, kind="ExternalOutput").ap()
    tok_o = nc.dram_tensor("tok_s", [t_steps, B], dt.float32, kind="ExternalOutput").ap()

    rg = [list(range(NC))]

    with tile.TileContext(nc) as tc:
        with (
            tc.tile_pool(name="persist", bufs=1) as pp,
            tc.tile_pool(name="lg", bufs=3) as lp,
            tc.tile_pool(name="psum", bufs=3, space="PSUM") as psp,
            tc.tile_pool(name="psg", bufs=2, space="PSUM") as psg,
            tc.tile_pool(name="pst", bufs=2, space="PSUM") as pst,
            tc.tile_pool(name="dram", bufs=2, space="DRAM") as dp,
        ):
            # ---- persistent SBUF ----
            whhT_sb = pp.tile([128, KC * G4], dt.bfloat16)    # [p, (k g)]
            woutT_sb = pp.tile([128, KC * VS], dt.bfloat16)   # [p, (k v)]
            bias2_sb = pp.tile([2, VS], dt.bfloat16)
            ones2_sb = pp.tile([2, B], dt.bfloat16)
            ident_sb = pp.tile([B, B], dt.float32)
            cb_sb = pp.tile([B, NWIN], dt.float32)
            hT_sb = pp.tile([128, KC * B], dt.bfloat16)       # full h^T, [p, (k b)]
            hTs_sb = pp.tile([HS, B], dt.bfloat16)            # own h-slice^T
            h_sb = pp.tile([B, HS], dt.float32)
            c_sb = pp.tile([B, HS], dt.float32)
            ct_sb = pp.tile([B, HS], dt.float32)              # tanh(c)
            ifgo_sb = pp.tile([B, G4], dt.float32)
            ig_sb = pp.tile([B, HS], dt.float32)
            xg_sb = pp.tile([B, G4], dt.float32)              # gathered E_proj rows
            tok_sb = pp.tile([B, 1], dt.uint32)
            wmax_sb = pp.tile([B, NWIN], dt.float32)          # per-window max
            widx_sb = pp.tile([B, NWIN * 8], dt.uint32)       # per-window top8 idx
            widxf_sb = pp.tile([B, NWIN * 8], dt.float32)
            gidx_sb = pp.tile([B, NWIN], dt.float32)
            lt_sb = pp.tile([B, NWIN], dt.float32)
            pen_sb = pp.tile([B, NWIN], dt.float32)
            m1_sb = pp.tile([B, 1], dt.float32)
            gi1_sb = pp.tile([B, 1], dt.float32)
            agin_sb = pp.tile([B, 2], dt.float32)
            ag8_sb = pp.tile([B, 2 * NC], dt.float32)
            m2_sb = pp.tile([B, 1], dt.float32)
            lt8_sb = pp.tile([B, NC], dt.float32)
            pen8_sb = pp.tile([B, NC], dt.float32)
            tokf_sb = pp.tile([B, 1], dt.float32)

            # ---- init loads ----
            nc.sync.dma_start(whhT_sb[:].rearrange("p (k g) -> p k g", k=KC),
                              whhT.rearrange("(k p) g -> p k g", p=128))
            nc.sync.dma_start(woutT_sb[:].rearrange("p (k v) -> p k v", k=KC),
                              woutT.rearrange("(k p) v -> p k v", p=128))
            nc.sync.dma_start(bias2_sb[:], bias2)
            nc.sync.dma_start(ones2_sb[:], ones2)
            nc.sync.dma_start(ident_sb[:], ident)
            nc.sync.dma_start(cb_sb[:], cbase)
            nc.gpsimd.memset(tok_sb[:], SOS)
            nc.gpsimd.memset(hT_sb[:], 0.0)
            nc.gpsimd.memset(c_sb[:], 0.0)

            for t in range(t_steps):
                # -- gates: W_hh part (uses h^T of t-1; overlaps AG1 of prev step) --
                gps = psg.tile([B, G4], dt.float32, space="PSUM", tag="gps")
                for k in range(KC):
                    nc.tensor.matmul(
                        gps[:, :],
                        lhsT=hT_sb[:, k * B:(k + 1) * B],
                        rhs=whhT_sb[:, k * G4:(k + 1) * G4],
                        start=(k == 0), stop=(k == KC - 1),
                    )

                # -- gather E_proj[tok] (includes x@W_ih.T + b_gate) --
                nc.gpsimd.indirect_dma_start(
                    out=xg_sb[:],
                    out_offset=None,
                    in_=eproj,
                    in_offset=bass.IndirectOffsetOnAxis(ap=tok_sb[:, :1], axis=0),
                )
                # record token used this step (debug/inspection)
                nc.vector.tensor_copy(tokf_sb[:], tok_sb[:])
                nc.sync.dma_start(tok_o[t:t + 1, :].rearrange("one b -> b one"),
                                  tokf_sb[:])

                # -- gates = gather + hh_psum; nonlinearities --
                nc.vector.tensor_tensor(ifgo_sb[:], xg_sb[:], gps[:, :], op=Alu.add)
                nc.scalar.activation(ifgo_sb[:, 0:2 * HS], ifgo_sb[:, 0:2 * HS], Act.Sigmoid)
                nc.scalar.activation(ifgo_sb[:, 2 * HS:3 * HS], ifgo_sb[:, 2 * HS:3 * HS], Act.Tanh)
                nc.scalar.activation(ifgo_sb[:, 3 * HS:4 * HS], ifgo_sb[:, 3 * HS:4 * HS], Act.Sigmoid)
                # c = f*c + i*g ; h = o * tanh(c)
                nc.vector.tensor_tensor(ig_sb[:], ifgo_sb[:, 0:HS], ifgo_sb[:, 2 * HS:3 * HS], op=Alu.mult)
                nc.vector.tensor_tensor(c_sb[:], ifgo_sb[:, HS:2 * HS], c_sb[:], op=Alu.mult)
                nc.vector.tensor_tensor(c_sb[:], c_sb[:], ig_sb[:], op=Alu.add)
                nc.scalar.activation(ct_sb[:], c_sb[:], Act.Tanh)
                nc.vector.tensor_tensor(h_sb[:], ifgo_sb[:, 3 * HS:4 * HS], ct_sb[:], op=Alu.mult)

                # -- h slice transpose -> bf16 -> AllGather -> full h^T --
                ptp = pst.tile([HS, B], dt.float32, space="PSUM", tag="ptp")
                nc.tensor.transpose(ptp[:], h_sb[:], ident_sb[:])
                nc.vector.tensor_copy(hTs_sb[:], ptp[:])   # fp32 -> bf16 cast

                ag2i = dp.tile([HS, B], dt.bfloat16, tag="ag2i")
                ag2o = dp.tile([HID, B], dt.bfloat16, tag="ag2o")
                nc.sync.dma_start(ag2i[:], hTs_sb[:])
                nc.gpsimd.collective_compute(
                    "AllGather", Alu.bypass, replica_groups=rg,
                    ins=[ag2i[:].opt()], outs=[ag2o[:].opt()],
                )
                nc.sync.dma_start(hT_sb[:].rearrange("p (k b) -> p k b", k=KC),
                                  ag2o[:].rearrange("(k p) b -> p k b", p=128))

                # -- logits: bf16 matmul over 13 N-windows --
                for j in range(NWIN):
                    w0 = j * WIN
                    nj = min(WIN, VS - w0)
                    ps = psp.tile([B, WIN], dt.float32, space="PSUM", tag="lgps")
                    for k in range(KC):
                        nc.tensor.matmul(
                            ps[:, :nj],
                            lhsT=hT_sb[:, k * B:(k + 1) * B],
                            rhs=woutT_sb[:, k * VS + w0:k * VS + w0 + nj],
                            start=(k == 0), stop=False,
                        )
                    # bias rows (hi+lo) via ones-columns
                    nc.tensor.matmul(
                        ps[:, :nj],
                        lhsT=ones2_sb[:],
                        rhs=bias2_sb[:, w0:w0 + nj],
                        start=False, stop=True,
                    )
                    lsb = lp.tile([B, WIN], dt.float32, tag="lsb")
                    nc.scalar.activation(lsb[:, :nj], ps[:, :nj], Act.Copy)
                    nc.vector.tensor_reduce(wmax_sb[:, j:j + 1], lsb[:, :nj],
                                            axis=mybir.AxisListType.X, op=Alu.max)
                    nc.sync.dma_start(logits_o[:, t, w0:w0 + nj], lsb[:, :nj])
                    if t == t_steps - 1:
                        continue
                    nc.vector.max_index(
                        widx_sb[:, 8 * j:8 * j + 8],
                        wmax_sb[:, j:j + 1].to_broadcast([B, 8]),
                        lsb[:, :nj],
                    )

                if t == t_steps - 1:
                    break

                # -- local argmax combine over windows --
                nc.vector.tensor_reduce(m1_sb[:], wmax_sb[:], axis=mybir.AxisListType.X, op=Alu.max)
                nc.vector.tensor_copy(widxf_sb[:], widx_sb[:])   # u32 -> f32
                nc.vector.tensor_tensor(
                    gidx_sb[:],
                    widxf_sb[:].rearrange("b (j e) -> b j e", e=8)[:, :, 0:1],
                    cb_sb[:], op=Alu.add)
                nc.vector.tensor_scalar(lt_sb[:], wmax_sb[:], m1_sb[:, :1], scalar2=None, op0=Alu.is_lt)
                nc.vector.scalar_tensor_tensor(pen_sb[:], lt_sb[:], 1e30, gidx_sb[:], op0=Alu.mult, op1=Alu.add)
                nc.vector.tensor_reduce(gi1_sb[:], pen_sb[:], axis=mybir.AxisListType.X, op=Alu.min)
                nc.vector.tensor_copy(agin_sb[:, 0:1], m1_sb[:])
                nc.vector.tensor_copy(agin_sb[:, 1:2], gi1_sb[:])

                # -- AllGather candidates; global argmax -> next token --
                ag1i = dp.tile([B, 2], dt.float32, tag="ag1i")
                ag1o = dp.tile([NC * B, 2], dt.float32, tag="ag1o")
                nc.sync.dma_start(ag1i[:], agin_sb[:])
                nc.gpsimd.collective_compute(
                    "AllGather", Alu.bypass, replica_groups=rg,
                    ins=[ag1i[:].opt()], outs=[ag1o[:].opt()],
                )
                nc.sync.dma_start(ag8_sb[:].rearrange("b (r two) -> b r two", two=2),
                                  ag1o[:].rearrange("(r b) two -> b r two", r=NC))
                v8 = ag8_sb[:].rearrange("b (r two) -> b two r", two=2)[:, 0:1, :]
                i8 = ag8_sb[:].rearrange("b (r two) -> b two r", two=2)[:, 1:2, :]
                nc.vector.tensor_reduce(m2_sb[:], v8, axis=mybir.AxisListType.X, op=Alu.max)
                nc.vector.tensor_scalar(lt8_sb[:], v8, m2_sb[:, :1], scalar2=None, op0=Alu.is_lt)
                nc.vector.scalar_tensor_tensor(pen8_sb[:], lt8_sb[:], 1e30, i8, op0=Alu.mult, op1=Alu.add)
                nc.vector.tensor_reduce(tokf_sb[:], pen8_sb[:], axis=mybir.AxisListType.X, op=Alu.min)
                nc.vector.tensor_copy(tok_sb[:], tokf_sb[:])    # f32 -> u32

            # ---- final state out ----
            nc.sync.dma_start(h_o, h_sb[:])
            nc.sync.dma_start(c_o, c_sb[:])

    nc.compile()
    return nc


def _prep_inputs(inputs):
    emb = np.asarray(inputs["emb_table"], F32)
    W_ih = np.asarray(inputs["W_ih"], F32)
    W_hh = np.asarray(inputs["W_hh"], F32)
    b_ih = np.asarray(inputs["b_ih"], F32)
    b_hh = np.asarray(inputs["b_hh"], F32)
    W_out = np.asarray(inputs["W_out"], F32)
    b_out = np.asarray(inputs["b_out"], F32)

    b_gate = b_ih + b_hh
    # E = emb @ W_ih.T + b_gate  (fp32, host): [VOCAB, 3072]
    E = emb @ W_ih.T + b_gate

    W_out_pad = np.zeros((VPAD, HID), F32)
    W_out_pad[:VOCAB] = W_out
    b_out_pad = np.full((VPAD,), NEG, F32)
    b_out_pad[:VOCAB] = b_out

    ident = np.eye(B, dtype=F32)
    ones2 = np.ones((2, B), BF16)

    in_maps = []
    for c in range(NC):
        rows = np.concatenate([np.arange(HS * c, HS * c + HS) + HID * g for g in range(4)])
        eproj_c = np.ascontiguousarray(E[:, rows])                     # [VOCAB, 384]
        whhT_c = np.ascontiguousarray(W_hh[rows].T.astype(BF16))       # [768, 384]
        wsh = W_out_pad[c * VS:(c + 1) * VS]                           # [VS, 768]
        woutT_c = np.ascontiguousarray(wsh.T.astype(BF16))             # [768, VS]
        bsh = b_out_pad[c * VS:(c + 1) * VS]
        bhi = bsh.astype(BF16)
        blo = (bsh - bhi.astype(F32)).astype(BF16)
        bias2_c = np.ascontiguousarray(np.stack([bhi, blo]))           # [2, VS]
        cb_c = np.broadcast_to(
            (c * VS + WIN * np.arange(NWIN, dtype=F32))[None, :], (B, NWIN))
        in_maps.append({
            "eproj": eproj_c,
            "whhT": whhT_c,
            "woutT": woutT_c,
            "bias2": bias2_c,
            "ones2": ones2,
            "ident": ident,
            "cbase": np.ascontiguousarray(cb_c.astype(F32)),
        })
    return in_maps


def run(inputs, t_steps=T, trace=False, trace_kwargs=None):
    from concourse.bass_utils import run_bass_kernel_spmd

    key = t_steps
    if key not in _cache:
        _cache[key] = _build(t_steps)
    nc = _cache[key]
    in_maps = _prep_inputs(inputs)
    kw = {}
    if trace:
        kw["trace"] = True
        if trace_kwargs:
            kw.update(trace_kwargs)
    res = run_bass_kernel_spmd(nc, in_maps, core_ids=list(range(NC)), **kw)

    logits = np.concatenate([r["logits_s"] for r in res.results], axis=2)[:, :, :VOCAB]
    h = np.concatenate([r["h_s"] for r in res.results], axis=1)[None]
    c = np.concatenate([r["c_s"] for r in res.results], axis=1)[None]
    return (logits, h, c), res


def kernel(**inputs):
    (logits, h, c), _ = run(inputs)
    return logits, h, c
